# revision 15
# baseline (speedup 1.0000x reference)
"""Butterfly depthwise-conv kernel for 8 Trainium2 NeuronCores.

Sharding: data-parallel over batch (B=8 -> 1 sample per core). Inside a core:
partitions = (channel, H-half): p<64 -> channel p rows [0, H/2); p>=64 ->
channel p-64 rows [H/2, H). Free axis = padded rows of one half:
row stride W+2, 2 left-pad cols, 1 halo row above/below the interior
(plus one extra zero guard row so the dx=+1 tap's row-wrap read of the
bottom halo row stays in bounds).

Per stage: the 9 depthwise taps are k=64 matmuls whose lhsT packs BOTH
branches (m=128: cols 0-63 branch a diag(w0), cols 64-127 branch b with the
butterfly mask folded in). Half-A and half-B matmuls run concurrently on
disjoint PE row-groups. PSUM accumulates the 9 taps; eviction does
relu(psum + bias) per branch (ACT/DVE, using cross-partition PSUM reads)
and a partition-aligned DVE add. Stage 5 fuses the +x residual and streams
the fp32 result to HBM.
"""

import json
import sys

sys.path.insert(0, "/opt/trn_rl_repo")

import ml_dtypes
import numpy as np

import concourse.bass as bass
import concourse.mybir as mybir
from concourse.tile import TileContext
from concourse.bass_utils import run_bass_kernel_spmd

# ---------------------------------------------------------------------------
# Walrus in this container accepts at most ONE sem wait / update per
# instruction; Tile emits more. Rewrite the BIR JSON before serialization:
# hoist excess waits onto preceding same-engine NoOps and excess updates onto
# trailing same-engine NoOps (engine queues are FIFO; a NoOp's update fires
# after the preceding instruction completes).
_wsplit_counter = [0]


def _fresh_name():
    _wsplit_counter[0] += 1
    return f"I-wsplit-{_wsplit_counter[0]}"


def _nop(engine, debug, wait=None, update=None):
    return {
        "debug": debug,
        "engine": engine,
        "ins": [],
        "name": _fresh_name(),
        "opcode": "NoOp",
        "outs": [],
        "sync_info": {
            "on_update": [update] if update else [],
            "on_wait": [wait] if wait else [],
        },
    }


def _rewrite_bir(j):
    for fn in j["functions"]:
        for bb in fn["blocks"]:
            new_insts = []
            for inst in bb["instructions"]:
                si = inst.get("sync_info")
                pre, post = [], []
                if si:
                    waits = si.get("on_wait") or []
                    if len(waits) > 1:
                        for w in waits[:-1]:
                            pre.append(_nop(inst["engine"], inst.get("debug", 0), wait=w))
                        si["on_wait"] = [waits[-1]]
                    ups = si.get("on_update") or []
                    opc = inst.get("opcode", "")
                    if len(ups) > 1 and "DMA" not in opc and "Dma" not in opc:
                        for u in ups[1:]:
                            post.append(_nop(inst["engine"], inst.get("debug", 0), update=u))
                        si["on_update"] = ups[:1]
                new_insts.extend(pre)
                new_insts.append(inst)
                new_insts.extend(post)
            bb["instructions"] = new_insts
    return j


_orig_to_json_bytes = bass.Bass.to_json_bytes


def _patched_to_json_bytes(self, *a, **kw):
    raw = _orig_to_json_bytes(self, *a, **kw)
    return json.dumps(_rewrite_bir(json.loads(raw))).encode()


bass.Bass.to_json_bytes = _patched_to_json_bytes
# ---------------------------------------------------------------------------

C = 64
AF = mybir.ActivationFunctionType
ALU = mybir.AluOpType
FP8 = mybir.dt.float8e4
DR = mybir.MatmulPerfMode.DoubleRow

# tap pairs for DoubleRow matmuls: each entry is ((dy0, dx0), (dy1, dx1) | None)
# the second k-slice of the rhs reads at base + SW (one row down). The HW
# requires the k-tile step to be a multiple of 16 (checkMatmultPerfMode), so
# SW is padded to 272 and pairs must differ by dy exactly; the three dy=+1
# taps ride solo with a zero slice-1 (the dummy slice reads the guard rows).
TAP_PAIRS = [
    ((-1, -1), (0, -1)),
    ((-1, 0), (0, 0)),
    ((-1, 1), (0, 1)),
    ((1, -1), None),
    ((1, 0), None),
    ((1, 1), None),
]


def build_program(H, W, num_bf):
    """Emit the Bass program for one core (one batch sample)."""
    HALF = H // 2
    SW = W + 16  # padded row stride (16-aligned for the DoubleRow k-tile step)
    ROWS = HALF + 2  # interior + top/bottom halo rows
    L = (ROWS + 1) * SW  # + one zero guard row for the dy=+1 dummy slice read
    CPR = 512 // W  # interior rows per psum chunk (2 at W=256)
    GROUP_ROWS = 2 * CPR  # rows per evict group (2 chunks)
    n_groups = HALF // GROUP_ROWS
    assert HALF % GROUP_ROWS == 0
    NCOL = GROUP_ROWS * W  # eviction columns per group (1024 at W=256)
    NPAIR = len(TAP_PAIRS)
    PAIR_SZ = 2 * 128  # lhsT free elems per DoubleRow matmul

    nc = bass.Bass()
    xbf_ext = nc.declare_dram_parameter("xbf", [C, H, W], mybir.dt.bfloat16, isOutput=False)
    xpad_ext = nc.declare_dram_parameter(
        "xpad", [128, (HALF + 3) * (W + 16)], FP8, isOutput=False
    )
    wt_ext = nc.declare_dram_parameter(
        "lhsT", [C, num_bf * NPAIR * PAIR_SZ], FP8, isOutput=False
    )
    bias_ext = nc.declare_dram_parameter(
        "bias", [128, num_bf], mybir.dt.float32, isOutput=False
    )
    out_ext = nc.declare_dram_parameter("out", [C, H, W], mybir.dt.float32, isOutput=True)

    def interior(r):
        # free-axis element offset of interior row r (0-based), col 0
        return (r + 1) * SW + 2

    def rows_ap(tile, pslice, r0, nrows, base_off=0):
        """[pslice, nrows, W] view of interior rows r0..r0+nrows-1 (+base_off cols)."""
        o = interior(r0) + base_off
        v = tile[pslice, o : o + nrows * SW]
        return v.rearrange("p (r w) -> p r w", w=SW)[:, :, 0:W]

    with TileContext(nc) as tc:
        with (
            tc.tile_pool(name="state", bufs=1) as state,
            tc.tile_pool(name="evict", bufs=2) as evict,
            tc.tile_pool(name="res", bufs=2) as res,
            tc.tile_pool(name="psA", bufs=2, space="PSUM") as psum_a,
            tc.tile_pool(name="psB", bufs=2, space="PSUM") as psum_b,
        ):
            now0 = state.tile([128, L], FP8)
            now1 = state.tile([128, L], FP8)
            wt = state.tile([128, num_bf * NPAIR * PAIR_SZ], FP8)
            bias_t = state.tile([128, num_bf], mybir.dt.float32)

            # zero the whole second state buffer once: stages only rewrite the
            # interior cols 2..2+W-1, so pads/halos/guard stay zero after this
            # (now0 arrives fully pre-padded from the host)
            nc.vector.memset(now1[:, :], 0.0)

            # weights: same data on partitions 0-63 and 64-127; split per
            # stage so stage-0 matmuls only wait for their own slice
            nc.sync.dma_start(out=bias_t[:], in_=bias_ext[:])
            SL = NPAIR * PAIR_SZ
            for st in range(num_bf):
                for pslice in (slice(0, 64), slice(64, 128)):
                    nc.gpsimd.dma_start(
                        out=wt[pslice, st * SL : (st + 1) * SL],
                        in_=wt_ext[:, st * SL : (st + 1) * SL],
                    )

            # initial load: host-prepadded bf16 x, fully contiguous DMAs
            CHUNK_ROWS = 16
            nrows_total = ROWS + 1
            r = 0
            while r < nrows_total:
                r1 = min(r + (8 if r == 0 else CHUNK_ROWS), nrows_total)
                if nrows_total - r1 < 4:
                    r1 = nrows_total
                nc.sync.dma_start(
                    out=now0[:, r * SW : r1 * SW],
                    in_=xpad_ext[:, r * SW : r1 * SW],
                )
                r = r1

            bufs = [now0, now1]
            for i in range(num_bf):
                src = bufs[i % 2]
                dst = bufs[(i + 1) % 2]
                last = i == num_bf - 1
                ba = bias_t[0:64, i : i + 1]
                bb = bias_t[64:128, i : i + 1]
                src_flat = src[:, :]
                for g in range(n_groups):
                    ps_a = psum_a.tile([128, 1024], mybir.dt.float32, tag="ps_a")
                    ps_b = psum_b.tile([128, 1024], mybir.dt.float32)
                    for cp in range(2):
                        r0 = g * GROUP_ROWS + cp * CPR
                        for q, ((dy0, dx0), t1) in enumerate(TAP_PAIRS):
                            woff = (i * NPAIR + q) * PAIR_SZ
                            base = interior(r0 + dy0) + dx0
                            for ps, pb in ((ps_a, 0), (ps_b, 64)):
                                rhs = bass.AP(
                                    src_flat.tensor,
                                    pb * L + base,
                                    [[L, 64], [SW, 2], [SW, CPR], [1, W]],
                                )
                                po = ps[:, cp * 512 : (cp + 1) * 512]
                                po = po.rearrange("p (r w) -> p r w", w=W)
                                lhsT = wt[
                                    pb : pb + 64, woff : woff + PAIR_SZ
                                ].rearrange("p (s m) -> p s m", s=2)
                                nc.tensor.matmul(
                                    po,
                                    lhsT,
                                    rhs,
                                    start=(q == 0),
                                    stop=(q == NPAIR - 1),
                                    perf_mode=DR,
                                )
                    # ---- eviction of GROUP_ROWS rows per half ----
                    # One full-width ACT relu per psum (both branches at once),
                    # then a cross-partition DVE copy aligns the branch-b half
                    # with branch-a, and one TT adds them.
                    u_a = evict.tile([128, NCOL], mybir.dt.bfloat16, tag="u_a")
                    u_b = evict.tile([128, NCOL], mybir.dt.bfloat16, tag="u_b")
                    t_a = evict.tile([64, NCOL], mybir.dt.bfloat16, tag="t_a")
                    t_b = evict.tile([128, NCOL], mybir.dt.bfloat16, tag="t_b")
                    bf = bias_t[:, i : i + 1]
                    # relu+bias split: ACT does u_a and the tail of u_b; DVE
                    # does the head of u_b via fused (psum+bias, max 0)
                    SPL = 768
                    nc.scalar.activation(u_a[:, :], ps_a[:, 0:NCOL], AF.Relu, bias=bf, scale=1.0)
                    nc.vector.tensor_scalar(
                        u_b[:, 0:SPL], ps_b[:, 0:SPL], bf, 0.0, ALU.add, ALU.max
                    )
                    nc.scalar.activation(
                        u_b[:, SPL:NCOL], ps_b[:, SPL:NCOL], AF.Relu, bias=bf, scale=1.0
                    )
                    if not last:
                        # cross-partition align copies ride the DMA engines
                        nc.sync.dma_start(out=t_a[:, :], in_=u_a[64:128, :])
                        nc.sync.dma_start(out=t_b[64:128, :], in_=u_b[0:64, :])
                    r0 = g * GROUP_ROWS
                    if not last:
                        nc.vector.tensor_add(
                            rows_ap(dst, slice(0, 64), r0, GROUP_ROWS),
                            u_a[0:64, :].rearrange("p (r w) -> p r w", w=W),
                            t_a[:, :].rearrange("p (r w) -> p r w", w=W),
                        )
                        nc.gpsimd.tensor_add(
                            rows_ap(dst, slice(64, 128), r0, GROUP_ROWS),
                            u_b[64:128, :].rearrange("p (r w) -> p r w", w=W),
                            t_b[64:128, :].rearrange("p (r w) -> p r w", w=W),
                        )
                        if g == 0:
                            # halfB row 0 -> halfA bottom halo (cross-out TT)
                            nc.vector.tensor_add(
                                dst[0:64, interior(HALF) : interior(HALF) + W],
                                u_b[64:128, 0:W],
                                t_b[64:128, 0:W],
                            )
                        if g == n_groups - 1:
                            # halfA last row -> halfB top halo (cross-out TT)
                            lo = (GROUP_ROWS - 1) * W
                            nc.vector.tensor_add(
                                dst[64:128, interior(-1) : interior(-1) + W],
                                u_a[0:64, lo : lo + W],
                                t_a[:, lo : lo + W],
                            )
                    else:
                        # final stage: x loaded into the OPPOSITE partition
                        # half, so the cross-half align step becomes
                        # t = relu_b + x (inputs share a base partition, only
                        # the out crosses); then og = relu_a + t in fp32 and a
                        # plain HWDGE store.
                        xr = res.tile([128, NCOL], mybir.dt.bfloat16, tag="xr", bufs=4)
                        og = res.tile([128, NCOL], mybir.dt.float32, tag="og")
                        nc.sync.dma_start(
                            out=xr[64:128, :].rearrange("p (r w) -> p r w", w=W),
                            in_=xbf_ext[:, r0 : r0 + GROUP_ROWS, :],
                        )
                        nc.sync.dma_start(
                            out=xr[0:64, :].rearrange("p (r w) -> p r w", w=W),
                            in_=xbf_ext[:, HALF + r0 : HALF + r0 + GROUP_ROWS, :],
                        )
                        nc.vector.tensor_add(t_a[:, :], u_a[64:128, :], xr[64:128, :])
                        nc.vector.tensor_add(t_b[64:128, :], u_b[0:64, :], xr[0:64, :])
                        nc.vector.tensor_add(og[0:64, :], u_a[0:64, :], t_a[:, :])
                        nc.vector.tensor_add(og[64:128, :], u_b[64:128, :], t_b[64:128, :])
                        nc.sync.dma_start(
                            out=out_ext[:, r0 : r0 + GROUP_ROWS, :],
                            in_=og[0:64, :].rearrange("p (r w) -> p r w", w=W),
                        )
                        nc.sync.dma_start(
                            out=out_ext[:, HALF + r0 : HALF + r0 + GROUP_ROWS, :],
                            in_=og[64:128, :].rearrange("p (r w) -> p r w", w=W),
                        )
    return nc


def host_prep(weights, biases, masks, num_bf):
    """Fold the butterfly masks into DoubleRow tap-pair lhsT matrices.

    Layout: [C, num_bf, NPAIR, 2, 128] fp8 (channel-major) so the device DMA
    is contiguous per partition. Slice s of pair q holds the diag matrix of
    tap TAP_PAIRS[q][s] (branch a in m cols 0-63, branch b with the butterfly
    mask folded in at cols 64-127)."""
    npair = len(TAP_PAIRS)
    lhsT = np.zeros((num_bf, npair, 2, C, 128), dtype=np.float32)
    for i in range(num_bf):
        m = masks[i]
        for q, pair in enumerate(TAP_PAIRS):
            for s, tap in enumerate(pair):
                if tap is None:
                    continue
                dy, dx = tap[0] + 1, tap[1] + 1
                for c in range(C):
                    lhsT[i, q, s, c, c] = weights[i, 0, c, 0, dy, dx]
                    lhsT[i, q, s, m[c], 64 + c] = weights[i, 1, c, 0, dy, dx]
    lhsT = np.ascontiguousarray(
        lhsT.transpose(3, 0, 1, 2, 4)
        .reshape(C, num_bf * npair * 2 * 128)
        .astype(ml_dtypes.float8_e4m3)
    )
    bias = np.concatenate([biases[:, 0, :], biases[:, 1, :]], axis=1)  # [nb, 128]
    bias = np.ascontiguousarray(bias.T.astype(np.float32))  # [128, nb]
    return lhsT, bias


def _run(x_full, weights, biases, masks, H, W, num_bf, trace=False):
    nc = build_program(H, W, num_bf)
    lhsT, bias = host_prep(
        np.asarray(weights, dtype=np.float32),
        np.asarray(biases, dtype=np.float32),
        np.asarray(masks),
        num_bf,
    )
    n = x_full.shape[0]
    xbf = np.ascontiguousarray(x_full.astype(ml_dtypes.bfloat16))
    x8 = x_full.astype(ml_dtypes.float8_e4m3)
    # pre-padded SBUF-layout copy: [128 partitions, (HALF+3)*(W+16)]
    HALF, SW = H // 2, W + 16
    xpad = np.zeros((n, 128, HALF + 3, SW), dtype=ml_dtypes.float8_e4m3)
    xpad[:, 0:64, 1 : HALF + 1, 2 : 2 + W] = x8[:, :, 0:HALF, :]
    xpad[:, 64:128, 1 : HALF + 1, 2 : 2 + W] = x8[:, :, HALF:H, :]
    xpad[:, 0:64, HALF + 1, 2 : 2 + W] = x8[:, :, HALF, :]  # A bottom halo
    xpad[:, 64:128, 0, 2 : 2 + W] = x8[:, :, HALF - 1, :]  # B top halo
    xpad = np.ascontiguousarray(xpad.reshape(n, 128, (HALF + 3) * SW))
    in_maps = [
        {"xbf": xbf[b], "xpad": xpad[b], "lhsT": lhsT, "bias": bias}
        for b in range(n)
    ]
    r = run_bass_kernel_spmd(nc, in_maps, core_ids=list(range(n)), trace=trace)
    out = np.stack([r.results[b]["out"] for b in range(n)], axis=0)
    return out, r


def kernel(x, weights, biases, masks):
    x = np.asarray(x, dtype=np.float32)
    out, _ = _run(x, weights, biases, masks, H=256, W=256, num_bf=6)
    return out



# revision 25
# speedup vs baseline: 1.3193x; 1.3193x over previous
"""Butterfly depthwise-conv kernel for 8 Trainium2 NeuronCores.

Sharding: data-parallel over batch (B=8 -> 1 sample per core). Inside a core:
partitions = (channel, H-half): p<64 -> channel p rows [0, H/2); p>=64 ->
channel p-64 rows [H/2, H). Free axis = padded rows of one half:
row stride W+2, 2 left-pad cols, 1 halo row above/below the interior
(plus one extra zero guard row so the dx=+1 tap's row-wrap read of the
bottom halo row stays in bounds).

Per stage: the 9 depthwise taps are k=64 matmuls whose lhsT packs BOTH
branches (m=128: cols 0-63 branch a diag(w0), cols 64-127 branch b with the
butterfly mask folded in). Half-A and half-B matmuls run concurrently on
disjoint PE row-groups. PSUM accumulates the 9 taps; eviction does
relu(psum + bias) per branch (ACT/DVE, using cross-partition PSUM reads)
and a partition-aligned DVE add. Stage 5 fuses the +x residual and streams
the fp32 result to HBM.
"""

import json
import sys

sys.path.insert(0, "/opt/trn_rl_repo")

import ml_dtypes
import numpy as np

import concourse.bass as bass
import concourse.mybir as mybir
from concourse.tile import TileContext
from concourse.bass_utils import run_bass_kernel_spmd

# ---------------------------------------------------------------------------
# Walrus in this container accepts at most ONE sem wait / update per
# instruction; Tile emits more. Rewrite the BIR JSON before serialization:
# hoist excess waits onto preceding same-engine NoOps and excess updates onto
# trailing same-engine NoOps (engine queues are FIFO; a NoOp's update fires
# after the preceding instruction completes).
_wsplit_counter = [0]


def _fresh_name():
    _wsplit_counter[0] += 1
    return f"I-wsplit-{_wsplit_counter[0]}"


def _nop(engine, debug, wait=None, update=None):
    return {
        "debug": debug,
        "engine": engine,
        "ins": [],
        "name": _fresh_name(),
        "opcode": "NoOp",
        "outs": [],
        "sync_info": {
            "on_update": [update] if update else [],
            "on_wait": [wait] if wait else [],
        },
    }


def _rewrite_bir(j):
    for fn in j["functions"]:
        for bb in fn["blocks"]:
            new_insts = []
            for inst in bb["instructions"]:
                si = inst.get("sync_info")
                pre, post = [], []
                if si:
                    waits = si.get("on_wait") or []
                    if len(waits) > 1:
                        for w in waits[:-1]:
                            pre.append(_nop(inst["engine"], inst.get("debug", 0), wait=w))
                        si["on_wait"] = [waits[-1]]
                    ups = si.get("on_update") or []
                    opc = inst.get("opcode", "")
                    if len(ups) > 1 and "DMA" not in opc and "Dma" not in opc:
                        for u in ups[1:]:
                            post.append(_nop(inst["engine"], inst.get("debug", 0), update=u))
                        si["on_update"] = ups[:1]
                new_insts.extend(pre)
                new_insts.append(inst)
                new_insts.extend(post)
            bb["instructions"] = new_insts
    return j


_orig_to_json_bytes = bass.Bass.to_json_bytes


def _patched_to_json_bytes(self, *a, **kw):
    raw = _orig_to_json_bytes(self, *a, **kw)
    return json.dumps(_rewrite_bir(json.loads(raw))).encode()


bass.Bass.to_json_bytes = _patched_to_json_bytes
# ---------------------------------------------------------------------------

C = 64
AF = mybir.ActivationFunctionType
ALU = mybir.AluOpType
FP8 = mybir.dt.float8e4
DR = mybir.MatmulPerfMode.DoubleRow

# DoubleRow tap pairs: the second k-slice of the rhs reads at base + SW (one
# row down). The HW requires the k-tile step to be a multiple of 16
# (checkMatmultPerfMode), so SW is padded to 272 and pairs must differ by dy
# exactly. The three dy=+1 taps run as PLAIN fp8 matmuls (512 streamed elems
# per 512 outputs vs DR's 1024, and they keep the fast FWL weight load).
TAP_PAIRS = [
    ((-1, -1), (0, -1)),
    ((-1, 0), (0, 0)),
    ((-1, 1), (0, 1)),
]
TAP_SOLOS = [(1, -1), (1, 0), (1, 1)]


def build_program(H, W, num_bf):
    """Emit the Bass program for one core (one batch sample)."""
    HALF = H // 2
    SW = W + 16  # padded row stride (16-aligned for the DoubleRow k-tile step)
    ROWS = HALF + 2  # interior + top/bottom halo rows
    L = (ROWS + 1) * SW  # + one zero guard row for the dy=+1 dummy slice read
    CPR = 512 // W  # interior rows per psum chunk (2 at W=256)
    GROUP_ROWS = 2 * CPR  # rows per evict group (2 chunks)
    n_groups = HALF // GROUP_ROWS
    assert HALF % GROUP_ROWS == 0
    NCOL = GROUP_ROWS * W  # eviction columns per group (1024 at W=256)
    NPAIR = len(TAP_PAIRS)
    NSOLO = len(TAP_SOLOS)
    PAIR_SZ = 2 * 128  # lhsT free elems per DoubleRow matmul
    STAGE_SZ = NPAIR * PAIR_SZ + NSOLO * 128  # lhsT elems per stage

    nc = bass.Bass()
    xbf_ext = nc.declare_dram_parameter("xbf", [C, H, W], mybir.dt.bfloat16, isOutput=False)
    xpad_ext = nc.declare_dram_parameter(
        "xpad", [128, (HALF + 3) * (W + 16)], FP8, isOutput=False
    )
    wt_ext = nc.declare_dram_parameter(
        "lhsT", [C, num_bf * STAGE_SZ], FP8, isOutput=False
    )
    bias_ext = nc.declare_dram_parameter(
        "bias", [128, num_bf], mybir.dt.float32, isOutput=False
    )
    out_ext = nc.declare_dram_parameter("out", [C, H, W], mybir.dt.bfloat16, isOutput=True)

    def interior(r):
        # free-axis element offset of interior row r (0-based), col 0
        return (r + 1) * SW + 2

    def rows_ap(tile, pslice, r0, nrows, base_off=0):
        """[pslice, nrows, W] view of interior rows r0..r0+nrows-1 (+base_off cols)."""
        o = interior(r0) + base_off
        v = tile[pslice, o : o + nrows * SW]
        return v.rearrange("p (r w) -> p r w", w=SW)[:, :, 0:W]

    with TileContext(nc) as tc:
        with (
            tc.tile_pool(name="state", bufs=1) as state,
            tc.tile_pool(name="evict", bufs=2) as evict,
            tc.tile_pool(name="res", bufs=2) as res,
            tc.tile_pool(name="psA", bufs=2, space="PSUM") as psum_a,
            tc.tile_pool(name="psB", bufs=2, space="PSUM") as psum_b,
        ):
            now0 = state.tile([128, L], FP8)
            now1 = state.tile([128, L], FP8)
            wt = state.tile([128, num_bf * STAGE_SZ], FP8)
            bias_t = state.tile([128, num_bf], mybir.dt.float32)

            # zero the whole second state buffer once: stages only rewrite the
            # interior cols 2..2+W-1, so pads/halos/guard stay zero after this
            # (now0 arrives fully pre-padded from the host)
            nc.vector.memset(now1[:, :], 0.0)

            # weights: same data on partitions 0-63 and 64-127; split per
            # stage so stage-0 matmuls only wait for their own slice
            nc.sync.dma_start(out=bias_t[:], in_=bias_ext[:])
            SL = STAGE_SZ
            for st in range(num_bf):
                for pslice in (slice(0, 64), slice(64, 128)):
                    nc.gpsimd.dma_start(
                        out=wt[pslice, st * SL : (st + 1) * SL],
                        in_=wt_ext[:, st * SL : (st + 1) * SL],
                    )

            # initial load: host-prepadded bf16 x, fully contiguous DMAs
            CHUNK_ROWS = 16
            nrows_total = ROWS + 1
            r = 0
            while r < nrows_total:
                r1 = min(r + (8 if r == 0 else CHUNK_ROWS), nrows_total)
                if nrows_total - r1 < 4:
                    r1 = nrows_total
                nc.sync.dma_start(
                    out=now0[:, r * SW : r1 * SW],
                    in_=xpad_ext[:, r * SW : r1 * SW],
                )
                r = r1

            bufs = [now0, now1]
            for i in range(num_bf):
                src = bufs[i % 2]
                dst = bufs[(i + 1) % 2]
                last = i == num_bf - 1
                ba = bias_t[0:64, i : i + 1]
                bb = bias_t[64:128, i : i + 1]
                src_flat = src[:, :]
                for g in range(n_groups):
                    ps_a = psum_a.tile([128, 1024], mybir.dt.float32, tag="ps_a")
                    ps_b = psum_b.tile([128, 1024], mybir.dt.float32)
                    for cp in range(2):
                        r0 = g * GROUP_ROWS + cp * CPR
                        for q, ((dy0, dx0), _) in enumerate(TAP_PAIRS):
                            woff = i * STAGE_SZ + q * PAIR_SZ
                            base = interior(r0 + dy0) + dx0
                            for ps, pb in ((ps_a, 0), (ps_b, 64)):
                                rhs = bass.AP(
                                    src_flat.tensor,
                                    pb * L + base,
                                    [[L, 64], [SW, 2], [SW, CPR], [1, W]],
                                )
                                po = ps[:, cp * 512 : (cp + 1) * 512]
                                po = po.rearrange("p (r w) -> p r w", w=W)
                                lhsT = wt[
                                    pb : pb + 64, woff : woff + PAIR_SZ
                                ].rearrange("p (s m) -> p s m", s=2)
                                nc.tensor.matmul(
                                    po,
                                    lhsT,
                                    rhs,
                                    start=(q == 0),
                                    stop=False,
                                    perf_mode=DR,
                                )
                        for sq, (dy0, dx0) in enumerate(TAP_SOLOS):
                            woff = i * STAGE_SZ + NPAIR * PAIR_SZ + sq * 128
                            base = interior(r0 + dy0) + dx0
                            for ps, pb in ((ps_a, 0), (ps_b, 64)):
                                rhs = bass.AP(
                                    src_flat.tensor,
                                    pb * L + base,
                                    [[L, 64], [SW, CPR], [1, W]],
                                )
                                po = ps[:, cp * 512 : (cp + 1) * 512]
                                po = po.rearrange("p (r w) -> p r w", w=W)
                                nc.tensor.matmul(
                                    po,
                                    wt[pb : pb + 64, woff : woff + 128],
                                    rhs,
                                    start=False,
                                    stop=(sq == NSOLO - 1),
                                )
                    # ---- eviction of GROUP_ROWS rows per half ----
                    # One full-width ACT relu per psum (both branches at once),
                    # then a cross-partition DVE copy aligns the branch-b half
                    # with branch-a, and one TT adds them.
                    u_a = evict.tile([128, NCOL], mybir.dt.bfloat16, tag="u_a")
                    u_b = evict.tile([128, NCOL], mybir.dt.bfloat16, tag="u_b")
                    t_a = evict.tile([64, NCOL], mybir.dt.bfloat16, tag="t_a")
                    t_b = evict.tile([128, NCOL], mybir.dt.bfloat16, tag="t_b")
                    bf = bias_t[:, i : i + 1]
                    nc.scalar.activation(u_a[:, :], ps_a[:, 0:NCOL], AF.Relu, bias=bf, scale=1.0)
                    nc.scalar.activation(u_b[:, :], ps_b[:, 0:NCOL], AF.Relu, bias=bf, scale=1.0)
                    if not last:
                        # bf16 pair-copies as int32 halves the DVE element count
                        nc.vector.tensor_copy(
                            out=t_a[:, :].bitcast(mybir.dt.int32),
                            in_=u_a[64:128, :].bitcast(mybir.dt.int32),
                        )
                        nc.vector.tensor_copy(
                            out=t_b[64:128, :].bitcast(mybir.dt.int32),
                            in_=u_b[0:64, :].bitcast(mybir.dt.int32),
                        )
                    r0 = g * GROUP_ROWS
                    if not last:
                        nc.vector.tensor_add(
                            rows_ap(dst, slice(0, 64), r0, GROUP_ROWS),
                            u_a[0:64, :].rearrange("p (r w) -> p r w", w=W),
                            t_a[:, :].rearrange("p (r w) -> p r w", w=W),
                        )
                        nc.vector.tensor_add(
                            rows_ap(dst, slice(64, 128), r0, GROUP_ROWS),
                            u_b[64:128, :].rearrange("p (r w) -> p r w", w=W),
                            t_b[64:128, :].rearrange("p (r w) -> p r w", w=W),
                        )
                        if g == 0:
                            # halfB row 0 -> halfA bottom halo (cross-out TT)
                            nc.vector.tensor_add(
                                dst[0:64, interior(HALF) : interior(HALF) + W],
                                u_b[64:128, 0:W],
                                t_b[64:128, 0:W],
                            )
                        if g == n_groups - 1:
                            # halfA last row -> halfB top halo (cross-out TT)
                            lo = (GROUP_ROWS - 1) * W
                            nc.vector.tensor_add(
                                dst[64:128, interior(-1) : interior(-1) + W],
                                u_a[0:64, lo : lo + W],
                                t_a[:, lo : lo + W],
                            )
                    else:
                        # final stage: x loaded into the OPPOSITE partition
                        # half, so the cross-half align step becomes
                        # t = relu_b + x (inputs share a base partition, only
                        # the out crosses); then og = relu_a + t in fp32 and a
                        # plain HWDGE store.
                        xr = res.tile([128, NCOL], mybir.dt.bfloat16, tag="xr", bufs=4)
                        og = res.tile([128, NCOL], mybir.dt.bfloat16, tag="og")
                        nc.sync.dma_start(
                            out=xr[64:128, :].rearrange("p (r w) -> p r w", w=W),
                            in_=xbf_ext[:, r0 : r0 + GROUP_ROWS, :],
                        )
                        nc.sync.dma_start(
                            out=xr[0:64, :].rearrange("p (r w) -> p r w", w=W),
                            in_=xbf_ext[:, HALF + r0 : HALF + r0 + GROUP_ROWS, :],
                        )
                        nc.vector.tensor_add(t_a[:, :], u_a[64:128, :], xr[64:128, :])
                        nc.vector.tensor_add(t_b[64:128, :], u_b[0:64, :], xr[0:64, :])
                        nc.vector.tensor_add(og[0:64, :], u_a[0:64, :], t_a[:, :])
                        nc.vector.tensor_add(og[64:128, :], u_b[64:128, :], t_b[64:128, :])
                        nc.sync.dma_start(
                            out=out_ext[:, r0 : r0 + GROUP_ROWS, :],
                            in_=og[0:64, :].rearrange("p (r w) -> p r w", w=W),
                        )
                        nc.sync.dma_start(
                            out=out_ext[:, HALF + r0 : HALF + r0 + GROUP_ROWS, :],
                            in_=og[64:128, :].rearrange("p (r w) -> p r w", w=W),
                        )
    return nc


def host_prep(weights, biases, masks, num_bf):
    """Fold the butterfly masks into DoubleRow tap-pair lhsT matrices.

    Layout: [C, num_bf, NPAIR, 2, 128] fp8 (channel-major) so the device DMA
    is contiguous per partition. Slice s of pair q holds the diag matrix of
    tap TAP_PAIRS[q][s] (branch a in m cols 0-63, branch b with the butterfly
    mask folded in at cols 64-127)."""
    npair = len(TAP_PAIRS)
    nsolo = len(TAP_SOLOS)
    nslot = 2 * npair + nsolo  # tap slots per stage (2 per DR pair, 1 per solo)
    lhsT = np.zeros((num_bf, nslot, C, 128), dtype=np.float32)
    for i in range(num_bf):
        m = masks[i]
        taps = [t for pair in TAP_PAIRS for t in pair] + list(TAP_SOLOS)
        for s, (dy, dx) in enumerate(taps):
            dy, dx = dy + 1, dx + 1
            for c in range(C):
                lhsT[i, s, c, c] = weights[i, 0, c, 0, dy, dx]
                lhsT[i, s, m[c], 64 + c] = weights[i, 1, c, 0, dy, dx]
    lhsT = np.ascontiguousarray(
        lhsT.transpose(2, 0, 1, 3)
        .reshape(C, num_bf * nslot * 128)
        .astype(ml_dtypes.float8_e4m3)
    )
    bias = np.concatenate([biases[:, 0, :], biases[:, 1, :]], axis=1)  # [nb, 128]
    bias = np.ascontiguousarray(bias.T.astype(np.float32))  # [128, nb]
    return lhsT, bias


def _run(x_full, weights, biases, masks, H, W, num_bf, trace=False):
    nc = build_program(H, W, num_bf)
    lhsT, bias = host_prep(
        np.asarray(weights, dtype=np.float32),
        np.asarray(biases, dtype=np.float32),
        np.asarray(masks),
        num_bf,
    )
    n = x_full.shape[0]
    xbf = np.ascontiguousarray(x_full.astype(ml_dtypes.bfloat16))
    x8 = x_full.astype(ml_dtypes.float8_e4m3)
    # pre-padded SBUF-layout copy: [128 partitions, (HALF+3)*(W+16)]
    HALF, SW = H // 2, W + 16
    xpad = np.zeros((n, 128, HALF + 3, SW), dtype=ml_dtypes.float8_e4m3)
    xpad[:, 0:64, 1 : HALF + 1, 2 : 2 + W] = x8[:, :, 0:HALF, :]
    xpad[:, 64:128, 1 : HALF + 1, 2 : 2 + W] = x8[:, :, HALF:H, :]
    xpad[:, 0:64, HALF + 1, 2 : 2 + W] = x8[:, :, HALF, :]  # A bottom halo
    xpad[:, 64:128, 0, 2 : 2 + W] = x8[:, :, HALF - 1, :]  # B top halo
    xpad = np.ascontiguousarray(xpad.reshape(n, 128, (HALF + 3) * SW))
    in_maps = [
        {"xbf": xbf[b], "xpad": xpad[b], "lhsT": lhsT, "bias": bias}
        for b in range(n)
    ]
    r = run_bass_kernel_spmd(nc, in_maps, core_ids=list(range(n)), trace=trace)
    out = np.stack(
        [np.asarray(r.results[b]["out"]).astype(np.float32) for b in range(n)], axis=0
    )
    return out, r


def kernel(x, weights, biases, masks):
    x = np.asarray(x, dtype=np.float32)
    out, _ = _run(x, weights, biases, masks, H=256, W=256, num_bf=6)
    return out

